# revision 1
# baseline (speedup 1.0000x reference)
"""Trainium2 Bass kernel for nn_Actor_att1 (gnn_message_passing).

Data-parallel over 8 NeuronCores: each core processes B/8 = 32768 rows.

Per-core pipeline (transposed activation layout [feature, batch], tiles of 512):
  - L1 of all 32 encoders (self + 15 other + 16 food) as ONE block-diagonal
    matmul group: W1_big [127, 1024], 8 matmuls of [127,128]x[127,512].
  - L2 similarly block-diagonal: 8 matmuls [128,64] -> enc_T [512 feat, 512 b].
  - Attention without softmax normalization: LayerNorm is scale-invariant, so
    unnormalized weights w_n = exp(score_n/4) suffice.  Score reduction over
    d (partition dim) and the weighted sums over n are PE "ones-matmuls";
    mean-centering of the attended vector is folded into the reduction matrix
    (RepC = blockdiag(I - 11^T/16)), so variance is just mean(C^2).
  - rstd = sqrt(1/(var+eps)) computed in a separate phase (ACT Sqrt lives in a
    different table set than Exp/Tanh -> 3 phases, 2 table switches total).
  - Final MLP in transposed layout, leaky-relu via ACT Lrelu, tanh via ACT.
  - Output transposed back to natural via PE, packed [128, 512] per core,
    un-permuted on the host.
"""

import numpy as np
import ml_dtypes

import concourse.bass as bass
import concourse.tile as tile
from concourse import mybir
from concourse.bass_utils import run_bass_kernel_spmd

F32 = mybir.dt.float32
BF16 = mybir.dt.bfloat16

N_CORES = 8
B_FULL = 262144
BC = B_FULL // N_CORES      # 32768 rows per core
OBS = 127
TB = 512                    # batch tile
NT = BC // TB               # 64 tiles
NSUB = 4                    # 128-row subtiles per tile
EPS = 1e-5

# ---- CONSTF32 column layout ----
W1_C = 0            # [0:127, 0:1024]  block-diag L1 weights
EYE_C = 1024        # [128,128] f32 identity
B1BIG_C = 1152      # 8 cols, [128,1] each: L1 bias per block
B2SB_C = 1160       # 4 cols: L2 bias per psum pair
B1M_C = 1164        # rows 0:32  final-MLP b1
B2M_C = 1165        # rows 0:32  final-MLP b2
B3M_C = 1166        # rows 0:2   final-MLP b3
F32_COLS = 1168

# ---- CONSTB (bf16) column layout ----
W2_C = 0            # [128, 1024]: 8 split-K blocks of [128,128]
EYEB_C = 1024       # [128,128] bf16 identity
SCORE_C = 1152      # 4 blocks [128,32]
REPC_C = 1280       # 4 blocks [128,32]
REPSELF_C = 1408    # [0:16, 128]
REPW_C = 1536       # 4 blocks [0:32, 128]
SQONES_C = 2048     # [0:32, 2]
M1SELF_C = 2050     # [0:16, 32]
M1REST_C = 2082     # [0:32, 32]
MW2_C = 2114        # [0:32, 32]
MW3_C = 2146        # [0:32, 2]
REPC48_C = 2176     # 4 blocks [128,48]: centered numerators + self identity
BF_COLS = 2368

_BASS_CACHE = {}


def _pack_consts(p):
    """Host-side packing of all weights into two constant arrays."""
    cf = np.zeros((128, F32_COLS), np.float32)
    cb = np.zeros((128, BF_COLS), np.float32)

    # --- W1 block-diag [127, 1024] + b1big [1024] ---
    w1 = np.zeros((127, 1024), np.float32)
    b1 = np.zeros(1024, np.float32)
    # agent 0: self  (input cols 0:4)
    w1[0:4, 0:32] = p['en_w1']
    b1[0:32] = p['en_b1']
    for i in range(15):               # other agents, input col map
        c = 32 + 32 * i
        w1[4 + 2 * i, c:c + 32] = p['oa_w1'][0]
        w1[5 + 2 * i, c:c + 32] = p['oa_w1'][1]
        w1[34 + 2 * i, c:c + 32] = p['oa_w1'][2]
        w1[35 + 2 * i, c:c + 32] = p['oa_w1'][3]
        w1[64 + i, c:c + 32] = p['oa_w1'][4]
        b1[c:c + 32] = p['oa_b1']
    for j in range(16):               # food agents
        c = 512 + 32 * j
        for k in range(3):
            w1[79 + 3 * j + k, c:c + 32] = p['g_w1'][k]
        b1[c:c + 32] = p['g_b1']
    cf[0:127, W1_C:W1_C + 1024] = w1
    cf[0:128, EYE_C:EYE_C + 128] = np.eye(128, dtype=np.float32)
    cf[:, B1BIG_C:B1BIG_C + 8] = b1.reshape(8, 128).T

    # --- W2 block-diag: 8 blocks [128, 64] ---
    w2s = [p['en_w2']] + [p['oa_w2']] * 15 + [p['g_w2']] * 16
    b2s = [p['en_b2']] + [p['oa_b2']] * 15 + [p['g_b2']] * 16
    w2big = np.zeros((128, 1024), np.float32)
    b2big = np.zeros(512, np.float32)
    for a in range(32):
        g, al = a // 4, a % 4        # g = h1 block, al = agent-in-block
        jj = a // 8                   # psum pair
        w2big[32 * al:32 * al + 32,
              128 * g + 16 * (a - 8 * jj):128 * g + 16 * (a - 8 * jj) + 16] = w2s[a]
        b2big[16 * a:16 * a + 16] = b2s[a]
    cb[:, W2_C:W2_C + 1024] = w2big
    cf[:, B2SB_C:B2SB_C + 4] = b2big.reshape(4, 128).T
    cb[:, EYEB_C:EYEB_C + 128] = np.eye(128, dtype=np.float32)

    # --- attention matrices, per feature-block j (agents 8j..8j+7) ---
    # score col for agent a: other (1..15) -> a-1 ; food (16..31) -> a-16+16
    for j in range(4):
        so = np.zeros((128, 32), np.float32)
        rc = np.zeros((128, 32), np.float32)
        rw = np.zeros((32, 128), np.float32)
        for nl in range(8):
            a = 8 * j + nl
            if a == 0:
                continue
            col = (a - 1) if a < 16 else (16 + a - 16)
            t = 0 if a < 16 else 1
            so[16 * nl:16 * nl + 16, col] = 1.0
            rw[col, 16 * nl:16 * nl + 16] = 1.0
            blk = np.eye(16, dtype=np.float32) - 1.0 / 16.0
            rc[16 * nl:16 * nl + 16, 16 * t:16 * t + 16] = blk
        cb[:, SCORE_C + 32 * j:SCORE_C + 32 * j + 32] = so
        cb[:, REPC_C + 32 * j:REPC_C + 32 * j + 32] = rc
        rc48 = np.zeros((128, 48), np.float32)
        rc48[:, 0:32] = rc
        if j == 0:
            rc48[np.arange(16), 32 + np.arange(16)] = 1.0  # self passthrough
            # score row 15 is never written -> exp(0)=1: route it to the
            # self rows of w_rep so products2 carries E_0's self unweighted
            rw[15, 0:16] = 1.0
        cb[:, REPC48_C + 48 * j:REPC48_C + 48 * j + 48] = rc48
        cb[0:32, REPW_C + 128 * j:REPW_C + 128 * j + 128] = rw
    rs = np.zeros((16, 128), np.float32)
    for k in range(8):
        rs[np.arange(16), 16 * k + np.arange(16)] = 1.0
    cb[0:16, REPSELF_C:REPSELF_C + 128] = rs
    sq = np.zeros((32, 2), np.float32)
    sq[0:16, 0] = 1.0 / 16.0
    sq[16:32, 1] = 1.0 / 16.0
    cb[0:32, SQONES_C:SQONES_C + 2] = sq

    # --- final MLP ---
    m_w1 = p['m_w1']  # [48, 32]; merged order [self, food, other]
    cb[0:16, M1SELF_C:M1SELF_C + 32] = m_w1[0:16]
    # M rows: 0-15 = other, 16-31 = food, 32-47 = self
    cb[0:48, M1REST_C:M1REST_C + 32] = np.concatenate(
        [m_w1[32:48], m_w1[16:32], m_w1[0:16]], axis=0)
    cb[0:32, MW2_C:MW2_C + 32] = p['m_w2']
    cb[0:32, MW3_C:MW3_C + 2] = p['m_w3']
    cf[0:32, B1M_C] = p['m_b1']
    cf[0:32, B2M_C] = p['m_b2']
    cf[0:2, B3M_C] = p['m_b3']

    # LN gain/bias are ones/zeros in setup_inputs; fold general case anyway:
    # out = relu(LN * g + b). We only support g==1, b==0 fast path; otherwise
    # fall back by folding g into rstd-mult (g per-dim requires a tensor op we
    # skip).  Assert instead.
    for k in ('oa_g', 'g_g'):
        assert np.allclose(p[k], 1.0), "LN gain != 1 unsupported"
    for k in ('oa_bln', 'g_bln'):
        assert np.allclose(p[k], 0.0), "LN bias != 0 unsupported"

    return cf, cb.astype(ml_dtypes.bfloat16)


def _split_multi_waits(nc):
    """This walrus build accepts only one sync-wait per instruction; move
    extra waits onto dedicated EventSemaphore instructions just before."""
    f = nc.m.functions[0]
    ctr = 0
    for blk in f.blocks:
        new_ins = []
        for ins in blk.instructions:
            si = getattr(ins, 'sync_info', None)
            ow = list(si.on_wait) if si is not None and si.on_wait else []
            if len(ow) > 1:
                for w in ow[:-1]:
                    ev = mybir.InstEventSemaphore(
                        name=f"wsplit_{ctr}",
                        engine=ins.engine,
                        ins=[], outs=[],
                        sync_info=mybir.SyncInfo(on_wait=[w], on_update=[]),
                    )
                    ctr += 1
                    new_ins.append(ev)
                si.on_wait = ow[-1:]
            new_ins.append(ins)
        blk.instructions[:] = new_ins
    return ctr


def _build_bass(nt=NT):
    nc = bass.Bass()
    s_in = nc.dram_tensor("s_in", [OBS, BC], F32, kind="ExternalInput")
    cfd = nc.dram_tensor("constf", [128, F32_COLS], F32, kind="ExternalInput")
    cbd = nc.dram_tensor("constb", [128, BF_COLS], BF16, kind="ExternalInput")
    out = nc.dram_tensor("out", [2, NT * TB], F32, kind="ExternalOutput")

    with tile.TileContext(nc) as tc:
        with (
            tc.tile_pool(name="singles", bufs=1) as singles,
            tc.tile_pool(name="xt", bufs=2) as xt_p,
            tc.tile_pool(name="h1", bufs=2) as h1_p,
            tc.tile_pool(name="enc", bufs=3) as enc_p,
            tc.tile_pool(name="work", bufs=3) as work_p,
            tc.tile_pool(name="pL", bufs=3, space="PSUM") as pL,
            tc.tile_pool(name="psm", bufs=1, space="PSUM") as psm
            , tc.tile_pool(name="p3", bufs=2, space="PSUM") as p3_p,
            tc.tile_pool(name="pacc", bufs=2, space="PSUM") as pacc,
        ):
            CF = singles.tile([128, F32_COLS], F32)
            CB = singles.tile([128, BF_COLS], BF16)
            nc.sync.dma_start(out=CF, in_=cfd[:, :])
            nc.sync.dma_start(out=CB, in_=cbd[:, :])
            eye = CF[:, EYE_C:EYE_C + 128]
            eyeb = CB[:, EYEB_C:EYEB_C + 128]

            # PE warm-up: make PE observe the const DMAs once, so later
            # matmuls carry at most one (fresh) DMA sync-wait each -- walrus
            # rejects Matmults with 2+ sync waits.
            scratch = singles.tile([1, 48], F32)
            dscratch = singles.tile([1, 8], F32)
            wf = psm.tile([128, 128], F32, tag="sm")
            nc.tensor.transpose(wf[0:128, 0:128], eye, eye)
            nc.vector.tensor_copy(out=scratch[0:1, 0:8], in_=wf[0:1, 0:8])
            wb = psm.tile([128, 128], BF16, tag="sm")
            nc.tensor.transpose(wb[0:128, 0:128], eyeb, eyeb)
            nc.vector.tensor_copy(out=scratch[0:1, 8:16], in_=wb[0:1, 0:8])
            # every compute engine observes both const DMAs once, so
            # steady-state instructions carry few sync waits
            nc.scalar.copy(out=scratch[0:1, 16:24], in_=CF[0:1, 0:8])
            nc.scalar.copy(out=scratch[0:1, 24:32], in_=CB[0:1, 0:8])
            nc.vector.tensor_copy(out=scratch[0:1, 32:40], in_=CF[0:1, 0:8])
            nc.vector.tensor_copy(out=scratch[0:1, 40:48], in_=CB[0:1, 0:8])

            rn_stage = singles.tile([128, NT * 192], BF16)
            var_stage = singles.tile([128, NT * 8], F32)
            rstd_stage = singles.tile([128, NT * 8], F32)

            # phase-1/phase-3 software pipeline: first half of phase 1,
            # its rstd, then phase 1 (second half) interleaved with
            # phase 3 (first half); only the two Sqrt ops switch tables.
            def phase1_body(t, _st):
                r0 = t * TB
                if t % 2 == 0:
                    xT2 = xt_p.tile([127, 2 * TB], F32, tag="xT")
                    _st['xT2'] = xT2
                    # absorber: Pool observes the PE WAR tick so the DMA
                    # itself carries only its lane wait (HW allows 1)
                    nc.gpsimd.memset(xT2[0:1, 0:4], 0.0)
                    nc.gpsimd.dma_start(
                        out=xT2, in_=s_in[:, r0:r0 + 2 * TB])
                    xT = _st['xT2'][:, 0:TB]
                else:
                    xT = _st['xT2'][:, TB:2 * TB]

                # L1 + L2 block-diagonal encoders
                h1t = []
                for g in range(8):
                    ps = pL.tile([128, TB], F32, tag="mm")
                    nc.tensor.matmul(
                        ps, CF[0:127, W1_C + 128 * g:W1_C + 128 * (g + 1)],
                        xT, start=True, stop=True)
                    hg = h1_p.tile([128, TB], BF16, tag=f"h1{g}")
                    bias = CF[:, B1BIG_C + g:B1BIG_C + g + 1]
                    if g % 2 == 0:
                        nc.scalar.activation(
                            out=hg, in_=ps,
                            func=mybir.ActivationFunctionType.Relu,
                            bias=bias, scale=1.0)
                    else:
                        nc.vector.tensor_scalar(
                            out=hg, in0=ps, scalar1=bias, scalar2=0.0,
                            op0=mybir.AluOpType.add, op1=mybir.AluOpType.max)
                    h1t.append(hg)

                E = []
                for jj in range(4):
                    ps = pL.tile([128, TB], F32, tag="mm")
                    for half in range(2):
                        g = 2 * jj + half
                        nc.tensor.matmul(
                            ps,
                            CB[:, W2_C + 128 * g:W2_C + 128 * (g + 1)],
                            h1t[g], start=(half == 0), stop=(half == 1))
                    ej = enc_p.tile([128, TB], BF16, tag=f"E{jj}")
                    bias = CF[:, B2SB_C + jj:B2SB_C + jj + 1]
                    if jj % 2 == 0:
                        nc.scalar.activation(
                            out=ej, in_=ps,
                            func=mybir.ActivationFunctionType.Relu,
                            bias=bias, scale=1.0)
                    else:
                        nc.vector.tensor_scalar(
                            out=ej, in0=ps, scalar1=bias, scalar2=0.0,
                            op0=mybir.AluOpType.add, op1=mybir.AluOpType.max)
                    E.append(ej)

                # self replicated across the 8 16-row groups
                srp = pL.tile([128, TB], F32, tag="mm")
                nc.tensor.matmul(srp, CB[0:16, REPSELF_C:REPSELF_C + 128],
                                 E[0][0:16, :], start=True, stop=True)
                sr = work_p.tile([128, TB], BF16, tag="sr")
                nc.scalar.copy(out=sr, in_=srp)

                # scores -> S [32, 512]
                S = pacc.tile([32, TB], F32, tag="acc")
                Pj_list = []
                for jj in range(4):
                    pj = work_p.tile([128, TB], BF16, tag=f"P{jj}")
                    if jj < 2:
                        nc.vector.tensor_mul(pj, E[jj], sr)
                    else:
                        nc.gpsimd.tensor_mul(pj, E[jj], sr)
                    Pj_list.append(pj)
                for jj in range(4):
                    nc.tensor.matmul(
                        S, CB[:, SCORE_C + 32 * jj:SCORE_C + 32 * (jj + 1)],
                        Pj_list[jj], start=(jj == 0), stop=(jj == 3))

                # w = exp(score / 4)
                wt = work_p.tile([32, TB], BF16, tag="wt")
                nc.scalar.activation(out=wt, in_=S,
                                     func=mybir.ActivationFunctionType.Exp,
                                     scale=0.25)

                # centered numerators + self passthrough, C [48, 512]
                C = pacc.tile([48, TB], F32, tag="acc")
                P2_list = []
                for jj in range(4):
                    wr = pL.tile([128, TB], F32, tag="mm")
                    nc.tensor.matmul(
                        wr, CB[0:32, REPW_C + 128 * jj:REPW_C + 128 * (jj + 1)],
                        wt, start=True, stop=True)
                    p2 = work_p.tile([128, TB], BF16, tag=f"P2{jj}")
                    nc.vector.tensor_mul(p2, E[jj], wr)
                    P2_list.append(p2)
                for jj in range(4):
                    nc.tensor.matmul(
                        C, CB[:, REPC48_C + 48 * jj:REPC48_C + 48 * (jj + 1)],
                        P2_list[jj], start=(jj == 0), stop=(jj == 3))

                rsb = work_p.tile([48, TB], BF16, tag="rsb")
                nc.scalar.activation(out=rsb, in_=C,
                                     func=mybir.ActivationFunctionType.Relu)
                sqb = work_p.tile([48, TB], BF16, tag="sqb")
                nc.scalar.activation(out=sqb, in_=C,
                                     func=mybir.ActivationFunctionType.Square)

                # var [128, 2] per subtile; +EPS folded into the drain
                vn = psm.tile([128, 128], F32, tag="sm")
                for s in range(NSUB):
                    nc.tensor.matmul(
                        vn[:, 2 * s:2 * s + 2],
                        sqb[:, 128 * s:128 * (s + 1)],
                        CB[0:48, SQONES_C:SQONES_C + 2],
                        start=True, stop=True)
                nc.vector.tensor_scalar(
                    out=var_stage[:, 8 * t:8 * t + 8], in0=vn[:, 0:8],
                    scalar1=EPS, scalar2=None, op0=mybir.AluOpType.add)

                # transpose relu'd numerators (+self) to natural, stage
                rn = psm.tile([128, 192], BF16, tag="sm")
                for s in range(NSUB):
                    nc.tensor.transpose(
                        rn[:, 48 * s:48 * s + 48],
                        rsb[:, 128 * s:128 * (s + 1)], eyeb[0:48, 0:48])
                nc.vector.tensor_copy(
                    out=rn_stage[:, 192 * t:192 * (t + 1)], in_=rn)

            def phase3_body(t):
                # scale LN cols by rstd; self cols pass through unscaled
                mn = work_p.tile([128, 192], BF16, tag="mn")
                for s in range(NSUB):
                    rsl = rstd_stage[:, 8 * t + 2 * s:8 * t + 2 * s + 2]
                    rb = bass.AP(tensor=rsl.tensor, offset=rsl.offset,
                                 ap=[rsl.ap[0], rsl.ap[1], [0, 16]])
                    nc.gpsimd.tensor_mul(
                        mn[:, 48 * s:48 * s + 32].rearrange(
                            "p (t2 d) -> p t2 d", t2=2),
                        rn_stage[:, 192 * t + 48 * s:192 * t + 48 * s + 32
                                 ].rearrange("p (t2 d) -> p t2 d", t2=2),
                        rb)
                    nc.gpsimd.tensor_copy(
                        out=mn[:, 48 * s + 32:48 * s + 48],
                        in_=rn_stage[:, 192 * t + 48 * s + 32:
                                     192 * t + 48 * s + 48])
                # transpose back: MT [48, 512]
                mt = p3_p.tile([48, TB], BF16, tag="p3")
                for s in range(NSUB):
                    nc.tensor.transpose(
                        mt[:, 128 * s:128 * (s + 1)],
                        mn[:, 48 * s:48 * s + 48], eyeb)
                msb = work_p.tile([48, TB], BF16, tag="msb")
                nc.vector.tensor_copy(out=msb, in_=mt)

                # final MLP (merged = [other, food, self] rows of msb)
                h1f = p3_p.tile([32, TB], F32, tag="p3")
                nc.tensor.matmul(h1f, CB[0:48, M1REST_C:M1REST_C + 32], msb,
                                 start=True, stop=True)
                hh1 = work_p.tile([32, TB], BF16, tag="hh1")
                nc.scalar.activation(out=hh1, in_=h1f,
                                     func=mybir.ActivationFunctionType.Lrelu,
                                     bias=CF[0:32, B1M_C:B1M_C + 1],
                                     scale=1.0, alpha=0.01)
                h2f = p3_p.tile([32, TB], F32, tag="p3")
                nc.tensor.matmul(h2f, CB[0:32, MW2_C:MW2_C + 32], hh1,
                                 start=True, stop=True)
                hh2 = work_p.tile([32, TB], BF16, tag="hh2")
                nc.scalar.activation(out=hh2, in_=h2f,
                                     func=mybir.ActivationFunctionType.Lrelu,
                                     bias=CF[0:32, B2M_C:B2M_C + 1],
                                     scale=1.0, alpha=0.01)
                of = p3_p.tile([32, TB], F32, tag="p3")
                nc.tensor.matmul(of[0:2, :], CB[0:32, MW3_C:MW3_C + 2], hh2,
                                 start=True, stop=True)
                osb = work_p.tile([2, TB], F32, tag="osb")
                nc.scalar.activation(out=osb, in_=of[0:2, :],
                                     func=mybir.ActivationFunctionType.Tanh,
                                     bias=CF[0:2, B3M_C:B3M_C + 1], scale=1.0)

                nc.gpsimd.tensor_copy(out=dscratch[0:1, 0:4],
                                      in_=osb[0:1, 508:512])
                nc.gpsimd.dma_start(out=out[:, TB * t:TB * (t + 1)], in_=osb)

            def rstd_chunk(c0, c1):
                nc.vector.reciprocal(out=rstd_stage[:, c0:c1],
                                     in_=var_stage[:, c0:c1])
                nc.scalar.activation(
                    out=rstd_stage[:, c0:c1], in_=rstd_stage[:, c0:c1],
                    func=mybir.ActivationFunctionType.Sqrt)

            _st = {}
            if nt < 4:
                for t in range(nt):
                    phase1_body(t, _st)
                rstd_chunk(0, 8 * nt)
                for t in range(nt):
                    phase3_body(t)
            else:
                NCH = 4 if nt % 4 == 0 else 2
                H = nt // NCH
                for c in range(NCH):
                    for t in range(c * H, (c + 1) * H):
                        phase1_body(t, _st)
                        if c > 0:
                            phase3_body(t - H)
                    rstd_chunk(8 * c * H, 8 * (c + 1) * H)
                for t in range((NCH - 1) * H, nt):
                    phase3_body(t)
    _split_multi_waits(nc)
    return nc


def kernel(**inputs):
    inputs = {k: np.asarray(v, np.float32) for k, v in inputs.items()}
    cf, cb = _pack_consts(inputs)

    if 'nc' not in _BASS_CACHE:
        _BASS_CACHE['nc'] = _build_bass()
    nc = _BASS_CACHE['nc']

    s = np.ascontiguousarray(inputs['s_input'])
    in_maps = []
    for i in range(N_CORES):
        in_maps.append({
            "s_in": np.ascontiguousarray(s[i * BC:(i + 1) * BC].T),
            "constf": cf,
            "constb": cb,
        })
    res = run_bass_kernel_spmd(nc, in_maps, core_ids=list(range(N_CORES)))
    outs = []
    for i in range(N_CORES):
        o = np.asarray(res.results[i]["out"])           # [2, BC]
        outs.append(np.ascontiguousarray(o.T))
    return np.concatenate(outs, axis=0)

